# revision 1
# baseline (speedup 1.0000x reference)
"""Trainium2 Bass kernel for DiagonalLinear: y = x * diag (elementwise scale of last axis).

x: [4, 4096, 4096] f32, diag: [4096] f32 -> y: [4, 4096, 4096] f32.

Data-parallel over 8 NeuronCores: the 16384 rows (batch*seq) are split into 8
shards of 2048 rows; diag is replicated. Each core streams its 32 MiB shard
through SBUF in T tiles of [128 partitions x F floats] (each partition holds
F/4096 consecutive rows), multiplies by a diag tile replicated across
partitions, and streams the result back out. Memory-bound: ~64 MiB of HBM
traffic per core.
"""

import numpy as np

import concourse.bass as bass
import concourse.bacc as bacc
import concourse.mybir as mybir
import concourse.tile as tile
from concourse.bass_utils import run_bass_kernel_spmd

BATCH, SEQ, SIZE = 4, 4096, 4096
N_CORES = 8
ROWS = BATCH * SEQ                   # 16384
ROWS_PER_CORE = ROWS // N_CORES      # 2048
P = 128                              # SBUF partitions
F = 8192                             # free-dim elements per partition per tile
ROWS_PER_PART = F // SIZE            # 2 consecutive rows per partition
T = ROWS_PER_CORE * SIZE // (P * F)  # 8 tiles of 4 MiB per core
FP32 = mybir.dt.float32

_built = None


def _build():
    """Build + schedule the per-core Tile kernel (same program on all 8 cores)."""
    nc = bacc.Bacc("TRN2", target_bir_lowering=False, debug=False)

    x = nc.dram_tensor("x", [T, P, F], FP32, kind="ExternalInput").ap()
    d = nc.dram_tensor("diag", [SIZE], FP32, kind="ExternalInput").ap()
    y = nc.dram_tensor("y", [T, P, F], FP32, kind="ExternalOutput").ap()

    with tile.TileContext(nc) as tc:
        with (
            tc.tile_pool(name="dpool", bufs=1) as dpool,
            tc.tile_pool(name="xpool", bufs=4) as xpool,
        ):
            # diag replicated across all 128 partitions, repeated F/SIZE times
            # along the free dim so it lines up with the rows packed per
            # partition. Step-0 partition axis on the DRAM source AP.
            dtile = dpool.tile([P, F], FP32)
            d_bcast = bass.AP(
                tensor=d.tensor,
                offset=d.offset,
                ap=[[0, P], [1, SIZE]],
            )
            for j in range(ROWS_PER_PART):
                nc.gpsimd.dma_start(
                    out=dtile[:, j * SIZE : (j + 1) * SIZE], in_=d_bcast
                )

            for t in range(T):
                xt = xpool.tile([P, F], FP32)
                nc.sync.dma_start(out=xt[:], in_=x[t])
                nc.vector.tensor_mul(xt[:], xt[:], dtile[:])
                nc.scalar.dma_start(out=y[t], in_=xt[:])

    nc.compile()
    return nc


def _get_nc():
    global _built
    if _built is None:
        _built = _build()
    return _built


def kernel(x: np.ndarray, diag: np.ndarray) -> np.ndarray:
    nc = _get_nc()
    xs = np.ascontiguousarray(np.asarray(x, dtype=np.float32)).reshape(
        N_CORES, T, P, F
    )
    dg = np.ascontiguousarray(np.asarray(diag, dtype=np.float32))
    in_maps = [{"x": xs[i], "diag": dg} for i in range(N_CORES)]
    res = run_bass_kernel_spmd(nc, in_maps, list(range(N_CORES)))
    out = np.stack([res.results[i]["y"] for i in range(N_CORES)])
    return out.reshape(BATCH, SEQ, SIZE)


# revision 2
# speedup vs baseline: 1.0515x; 1.0515x over previous
"""Trainium2 Bass kernel for DiagonalLinear: y = x * diag (elementwise scale of last axis).

x: [4, 4096, 4096] f32, diag: [4096] f32 -> y: [4, 4096, 4096] f32.

Data-parallel over 8 NeuronCores: the 16384 rows (batch*seq) are split into 8
shards of 2048 rows; diag is replicated. Each core streams its 32 MiB shard
through SBUF in T tiles of [128 partitions x F floats] (each partition holds
F/4096 consecutive rows), multiplies by a diag tile replicated across
partitions, and streams the result back out. Memory-bound: ~64 MiB of HBM
traffic per core.
"""

import numpy as np

import concourse.bass as bass
import concourse.bacc as bacc
import concourse.mybir as mybir
import concourse.tile as tile
from concourse.bass_utils import run_bass_kernel_spmd

BATCH, SEQ, SIZE = 4, 4096, 4096
N_CORES = 8
ROWS = BATCH * SEQ                   # 16384
ROWS_PER_CORE = ROWS // N_CORES      # 2048
P = 128                              # SBUF partitions
F = 8192                             # free-dim elements per partition per tile
ROWS_PER_PART = F // SIZE            # 2 consecutive rows per partition
T = ROWS_PER_CORE * SIZE // (P * F)  # 8 tiles of 4 MiB per core
FP32 = mybir.dt.float32

_built = None


def _build():
    """Build + schedule the per-core Tile kernel (same program on all 8 cores)."""
    nc = bacc.Bacc("TRN2", target_bir_lowering=False, debug=False)

    x = nc.dram_tensor("x", [T, P, F], FP32, kind="ExternalInput").ap()
    d = nc.dram_tensor("diag", [SIZE], FP32, kind="ExternalInput").ap()
    y = nc.dram_tensor("y", [T, P, F], FP32, kind="ExternalOutput").ap()

    with tile.TileContext(nc) as tc:
        with (
            tc.tile_pool(name="dpool", bufs=1) as dpool,
            tc.tile_pool(name="xpool", bufs=4) as xpool,
        ):
            # Load diag once (16 KiB) into partition 0, then replicate it
            # across all 128 partitions on-chip — no extra HBM traffic.
            d0 = dpool.tile([1, SIZE], FP32)
            nc.sync.dma_start(out=d0[:], in_=d[None, :])
            dtile = dpool.tile([P, SIZE], FP32)
            nc.gpsimd.partition_broadcast(dtile[:], d0[:])

            for t in range(T):
                xt = xpool.tile([P, F], FP32)
                nc.sync.dma_start(out=xt[:], in_=x[t])
                for j in range(ROWS_PER_PART):
                    sl = xt[:, j * SIZE : (j + 1) * SIZE]
                    nc.vector.tensor_mul(sl, sl, dtile[:])
                nc.scalar.dma_start(out=y[t], in_=xt[:])

    nc.compile()
    return nc


def _get_nc():
    global _built
    if _built is None:
        _built = _build()
    return _built


def kernel(x: np.ndarray, diag: np.ndarray) -> np.ndarray:
    nc = _get_nc()
    xs = np.ascontiguousarray(np.asarray(x, dtype=np.float32)).reshape(
        N_CORES, T, P, F
    )
    dg = np.ascontiguousarray(np.asarray(diag, dtype=np.float32))
    in_maps = [{"x": xs[i], "diag": dg} for i in range(N_CORES)]
    res = run_bass_kernel_spmd(nc, in_maps, list(range(N_CORES)))
    out = np.stack([res.results[i]["y"] for i in range(N_CORES)])
    return out.reshape(BATCH, SEQ, SIZE)
